# revision 1
# baseline (speedup 1.0000x reference)
"""Causal single-head attention on 8 Trainium2 NeuronCores, K/V pair-exchange.

Problem: x [4, 2048, 1024], w_q/w_k/w_v [1024, 1024] (nn.Linear convention,
y = x @ W.T). Computes q,k,v projections, causal softmax(q k^T / sqrt(D)) @ v.

Sharding: 2 cores per batch element. Core parity p owns token half
H_p = [p*1024, (p+1)*1024) and computes K^T/V for ONLY its half (halving the
duplicated projection work of the previous design); the halves are exchanged
between the pair via two fenced 2MB AllGathers (replica groups [[0,1],[2,3],
[4,5],[6,7]]). Concurrent pair-AGs corrupt data on the odd member (measured),
so the V-AG is fenced behind the K-AG by a data-dependency row in its bounce
buffer. Queries: parity-interleaved 128-tiles exactly as before (slot k has a
kv window of 256k tokens), host-gathered, projected to Q^T kept in SBUF.

All matmul operands are bf16 (full PE rate, exact-rate transposes, half the
DMA and SBUF of f32); softmax statistics and PSUM accumulation stay f32.
"""
import numpy as np
import ml_dtypes
from contextlib import ExitStack

import concourse.bass as bass
import concourse.tile as tile
import concourse.mybir as mybir
from concourse.bass_utils import run_bass_kernel_spmd
from concourse.masks import make_identity

# NOTE: unlike the f32r predecessor, no --enable-ldw-opt patch: bf16 matmuls
# get standalone InstLdweights from tile legalization (shadow-buffered on HW,
# zero marginal cost), and walrus's ldw-opt pass rejects standalone LDWs.

F32 = mybir.dt.float32
BF16 = mybir.dt.bfloat16
AF = mybir.ActivationFunctionType
AX = mybir.AxisListType

B, S, E, D = 4, 2048, 1024, 1024
NCORES = 8
NSLOT = 8              # slots k=1..8, kv window = 256*k tokens
NQ = NSLOT * 128       # queries per core
HT = S // 2            # tokens projected per core (own half)
EC = E // 128          # e-chunks
DC = D // 128          # d-chunks
SCALE = 1.0 / 32.0     # 1/sqrt(D)
MASKVAL = -30000.0
GROUPS = [[0, 1], [2, 3], [4, 5], [6, 7]]

_prog = None


def _split_multi_waits(nc, max_waits=1):
    """The walrus build in this container has one sync-wait slot per
    instruction; hoist extra waits onto preceding same-engine NoOps."""
    n = 0
    for f in nc.m.functions:
        for b in f.blocks:
            insts = b.instructions
            out = []
            changed = False
            for ins in insts:
                si = ins.sync_info
                if si is not None and len(si.on_wait) > max_waits:
                    waits = list(si.on_wait)
                    for w in waits[:-max_waits]:
                        nop = mybir.InstNoOp(name=f"I-waitsplit-{n}")
                        n += 1
                        nop.engine = ins.engine
                        nop.sync_info = mybir.SyncInfo(on_wait=[w], on_update=[])
                        out.append(nop)
                    ins.sync_info = mybir.SyncInfo(
                        on_wait=waits[-max_waits:], on_update=list(si.on_update))
                    changed = True
                out.append(ins)
            if changed:
                b.instructions = out
    return nc


def _build(split=True):
    nc = bass.Bass(trn_type="TRN2", target_bir_lowering=False, debug=False)
    xoT = nc.dram_tensor("xoT", [E, HT], BF16, kind="ExternalInput").ap()
    xqT = nc.dram_tensor("xqT", [E, NQ], BF16, kind="ExternalInput").ap()
    wqT = nc.dram_tensor("wqT", [E, D], BF16, kind="ExternalInput").ap()
    wkT = nc.dram_tensor("wkT", [E, D], BF16, kind="ExternalInput").ap()
    wvT = nc.dram_tensor("wvT", [E, D], BF16, kind="ExternalInput").ap()
    maskin = nc.dram_tensor("mask", [128, 256], F32, kind="ExternalInput").ap()
    out = nc.dram_tensor("out", [NQ, D], F32, kind="ExternalOutput").ap()

    # collective bounce/out buffers. Pair-AG cost ~ 10us floor + 13us/MB;
    # THREE or more concurrent pair-AGs corrupt the odd member but TWO are
    # safe (measured), so the 4 x 1MB AGs here are fenced two-deep: K halves
    # flow data-gated, Va waits K-a, Vb waits K-b. The extra row on the V
    # buffers carries the fence bytes.
    HH = HT // 2
    DQ = D // 2
    bncKq, gathKq = [], []
    for q in range(2):
        bncKq.append(nc.dram_tensor(f"bncK{q}", [DQ, HT], BF16).ap())
        gathKq.append(nc.dram_tensor(f"gathK{q}", [2, DQ, HT], BF16).ap())
    bncVa = nc.dram_tensor("bncVa", [HH + 1, D], BF16).ap()
    gathVa = nc.dram_tensor("gathVa", [2, HH + 1, D], BF16).ap()
    bncVb = nc.dram_tensor("bncVb", [HH + 1, D], BF16).ap()
    gathVb = nc.dram_tensor("gathVb", [2, HH + 1, D], BF16).ap()

    with tile.TileContext(nc) as tc, ExitStack() as ctx:
        const = ctx.enter_context(tc.tile_pool(name="const", bufs=1))
        ident = const.tile([128, 128], BF16)
        make_identity(nc, ident[:])
        mask_sb = const.tile([128, 256], F32)
        nc.sync.dma_start(mask_sb[:], maskin[:])

        # Q^T stays resident until the end of attention
        qtp = ctx.enter_context(tc.tile_pool(name="qtp", bufs=1))
        qts = [qtp.tile([128, NQ], BF16, name=f"qt{d}") for d in range(DC)]

        # ---- Phase 1: K_own^T -> bncK (AG), V_own -> bncV (AG), Q^T ----
        with tc.tile_pool(name="wp", bufs=1) as wp, \
             tc.tile_pool(name="xp", bufs=1) as xp, \
             tc.tile_pool(name="st", bufs=1) as stp, \
             tc.tile_pool(name="ps1", bufs=4, space="PSUM") as pp:
            wk = [wp.tile([128, D], BF16, name=f"wk{e}") for e in range(EC)]
            wv = [wp.tile([128, D], BF16, name=f"wv{e}") for e in range(EC)]
            wq = [wp.tile([128, D], BF16, name=f"wq{e}") for e in range(EC)]
            xo = [xp.tile([128, HT], BF16, name=f"xo{e}") for e in range(EC)]

            # startup: interleave wk + x chunks so matmuls start early
            for e in range(EC):
                nc.sync.dma_start(wk[e][:, :512], wkT[e * 128:(e + 1) * 128, :512])
                nc.sync.dma_start(xo[e][:], xoT[e * 128:(e + 1) * 128, :])
            for e in range(EC):
                nc.sync.dma_start(wk[e][:, 512:], wkT[e * 128:(e + 1) * 128, 512:])
            for e in range(EC):
                nc.sync.dma_start(wv[e][:], wvT[e * 128:(e + 1) * 128, :])
            for e in range(EC):
                nc.sync.dma_start(wq[e][:], wqT[e * 128:(e + 1) * 128, :])

            # K_own^T: d-half-outer so AG-K0 triggers after half of K_own;
            # both K AGs are data-gated only (two-deep in flight is safe)
            kown = [stp.tile([128, HT], BF16, name=f"ko{d}") for d in range(DC)]
            for q in range(2):
                for g in range(2):
                    psl = [pp.tile([128, 512], F32, name=f"pk{q}_{g}_{d}",
                                   tag="pp") for d in range(4)]
                    for e in range(EC):
                        for d in range(4):
                            dd = q * 4 + d
                            nc.tensor.matmul(psl[d][:],
                                             wk[e][:, dd * 128:(dd + 1) * 128],
                                             xo[e][:, g * 512:(g + 1) * 512],
                                             start=(e == 0), stop=(e == EC - 1))
                    for d in range(4):
                        dd = q * 4 + d
                        nc.vector.tensor_copy(kown[dd][:, g * 512:(g + 1) * 512],
                                              psl[d][:])
                        nc.scalar.dma_start(
                            bncKq[q][d * 128:(d + 1) * 128,
                                     g * 512:(g + 1) * 512],
                            kown[dd][:, g * 512:(g + 1) * 512])
                nc.gpsimd.collective_compute(
                    "AllGather", mybir.AluOpType.bypass, replica_groups=GROUPS,
                    ins=[bncKq[q].opt()], outs=[gathKq[q].opt()])

            # V_own: stationary x chunks, moving wv; token half v first
            vown = [stp.tile([128, D], BF16, name=f"vo{t}") for t in range(HT // 128)]
            bncVs = [bncVa, bncVb]
            for v in range(2):
                for tl in range(HH // 128):
                    t = v * (HH // 128) + tl
                    for h in range(2):
                        ps = pp.tile([128, 512], F32, name=f"pv{t}_{h}", tag="pp")
                        for e in range(EC):
                            nc.tensor.matmul(ps[:], xo[e][:, t * 128:(t + 1) * 128],
                                             wv[e][:, h * 512:(h + 1) * 512],
                                             start=(e == 0), stop=(e == EC - 1))
                        nc.vector.tensor_copy(vown[t][:, h * 512:(h + 1) * 512],
                                              ps[:])
                    nc.scalar.dma_start(bncVs[v][tl * 128:(tl + 1) * 128, :],
                                      vown[t][:])
                # fence: single DRAM->DRAM DMA carrying K-AG output bytes
                # into the V bounce so the V-AG trigger waits for it
                nc.scalar.dma_start(bncVs[v][HH:HH + 1, 0:16],
                                    gathKq[v][1, 0:1, 0:16])
                nc.gpsimd.collective_compute(
                    "AllGather", mybir.AluOpType.bypass, replica_groups=GROUPS,
                    ins=[bncVs[v].opt()],
                    outs=[(gathVa if v == 0 else gathVb).opt()])

            # Q^T: stationary wq chunks, moving xq
            xq = [xp.tile([128, NQ], BF16, name=f"xq{e}") for e in range(EC)]
            for e in range(EC):
                nc.sync.dma_start(xq[e][:], xqT[e * 128:(e + 1) * 128, :])
            for d in range(DC):
                for g in range(2):
                    ps = pp.tile([128, 512], F32, name=f"pq{d}_{g}", tag="pp")
                    for e in range(EC):
                        nc.tensor.matmul(ps[:], wq[e][:, d * 128:(d + 1) * 128],
                                         xq[e][:, g * 512:(g + 1) * 512],
                                         start=(e == 0), stop=(e == EC - 1))
                    nc.vector.tensor_copy(qts[d][:, g * 512:(g + 1) * 512], ps[:])

        # ---- Phase 2: load gathered K^T / V into SBUF ----
        # token order: [A:H0(0:1024); B:H1(1024:2048)], each half split into
        # two 512-token quarters carried by the a/b AGs
        kvp = ctx.enter_context(tc.tile_pool(name="kvp", bufs=1))
        kts = [kvp.tile([128, S], BF16, name=f"kt{d}") for d in range(DC)]
        vts = [kvp.tile([128, D], BF16, name=f"vt{t}") for t in range(S // 128)]
        for q in range(2):
            for d2 in range(4):
                d = q * 4 + d2
                for r in range(2):
                    nc.sync.dma_start(kts[d][:, r * HT:(r + 1) * HT],
                                      gathKq[q][r, d2 * 128:(d2 + 1) * 128, :])
        for t in range(S // 128):
            r, tl = divmod(t, HT // 128)
            v, tq = divmod(tl, HH // 128)
            src = gathVa if v == 0 else gathVb
            nc.sync.dma_start(vts[t][:], src[r, tq * 128:(tq + 1) * 128, :])

        # ---- Phase 3: attention, one slot per kv-length class ----
        slot_order = [8, 3, 7, 4, 6, 5, 2, 1]
        with tc.tile_pool(name="att", bufs=1) as ap_, \
             tc.tile_pool(name="ps3", bufs=1, space="PSUM") as pp3:
            for k in slot_order:
                kv = 256 * k
                nch = kv // 128
                ngr = (kv + 511) // 512

                # 4 bufs: deeper cross-slot S pipelining
                s_ps = [pp3.tile([128, 512], F32, name=f"sps{k}_{g}", tag="sps",
                                 bufs=4) for g in range(ngr)]
                for g in range(ngr):
                    w = min(512, kv - g * 512)
                    for d in range(DC):
                        nc.tensor.matmul(s_ps[g][:, :w],
                                         qts[d][:, (k - 1) * 128:k * 128],
                                         kts[d][:, g * 512:g * 512 + w],
                                         start=(d == 0), stop=(d == DC - 1))

                # psum -> sbuf copies (mask folded into the last 256 cols)
                # with per-group running max
                s_sb = ap_.tile([128, 2048], F32, name=f"s{k}", tag="s", bufs=2)
                mparts = ap_.tile([128, 4], F32, name=f"mp{k}", tag="mp", bufs=2)
                lg = ngr - 1
                lw = kv - lg * 512
                for g in range(lg):
                    nc.scalar.copy(s_sb[:, g * 512:(g + 1) * 512], s_ps[g][:])
                if lw == 512:
                    nc.scalar.copy(s_sb[:, kv - 512:kv - 256], s_ps[lg][:, :256])
                    nc.vector.tensor_add(s_sb[:, kv - 256:kv],
                                         s_ps[lg][:, 256:512], mask_sb[:])
                else:
                    nc.vector.tensor_add(s_sb[:, kv - 256:kv],
                                         s_ps[lg][:, :256], mask_sb[:])
                for g in range(ngr):
                    w = min(512, kv - g * 512)
                    nc.vector.reduce_max(mparts[:, g:g + 1],
                                         s_sb[:, g * 512:g * 512 + w], axis=AX.X)

                m = ap_.tile([128, 1], F32, name=f"m{k}", tag="m", bufs=2)
                nc.vector.reduce_max(m[:], mparts[:, :ngr], axis=AX.X)
                negm = ap_.tile([128, 1], F32, name=f"negm{k}", tag="negm", bufs=2)
                nc.scalar.mul(negm[:], m[:], -SCALE)
                p_sb = ap_.tile([128, 2048], BF16, name=f"p{k}", tag="p", bufs=2)
                lparts = ap_.tile([128, 4], F32, name=f"lp{k}", tag="lp", bufs=2)
                for g in range(ngr):
                    w = min(512, kv - g * 512)
                    nc.scalar.activation(p_sb[:, g * 512:g * 512 + w],
                                         s_sb[:, g * 512:g * 512 + w], AF.Exp,
                                         bias=negm[:], scale=SCALE,
                                         accum_out=lparts[:, g:g + 1])
                lsum = ap_.tile([128, 1], F32, name=f"lsum{k}", tag="lsum", bufs=2)
                nc.vector.reduce_sum(lsum[:], lparts[:, :ngr], axis=AX.X)
                linv = ap_.tile([128, 1], F32, name=f"linv{k}", tag="linv", bufs=2)
                nc.vector.reciprocal(linv[:], lsum[:])

                # per-slot pt buffer: all 8 slots' S/softmax/transpose chains
                # must drain while the V-AGs are in flight, so no rotation
                pt = ap_.tile([128, kv], BF16, name=f"pt{k}", tag=f"pt{k}",
                              bufs=1)
                for c in range(nch):
                    tps = pp3.tile([128, 128], BF16, name=f"tp{k}_{c}", tag="tps",
                                   bufs=2)
                    nc.tensor.transpose(tps[:], p_sb[:, c * 128:(c + 1) * 128],
                                        ident[:])
                    if c % 2 == 0:
                        nc.vector.tensor_copy(pt[:, c * 128:(c + 1) * 128], tps[:])
                    else:
                        nc.scalar.copy(pt[:, c * 128:(c + 1) * 128], tps[:])

                # O accumulation ordered by V-AG arrival (Va: c%8<4, then Vb)
                o_ps = [pp3.tile([128, 512], F32, name=f"op{k}_{h}", tag="ops",
                                 bufs=2) for h in range(2)]
                corder = ([c for c in range(nch) if c % 8 < 4]
                          + [c for c in range(nch) if c % 8 >= 4])
                for ci, c in enumerate(corder):
                    lhs = pt[:, c * 128:(c + 1) * 128]
                    for h in range(2):
                        nc.tensor.matmul(o_ps[h][:], lhs,
                                         vts[c][:, h * 512:(h + 1) * 512],
                                         start=(ci == 0), stop=(ci == nch - 1))

                o_sb = ap_.tile([128, D], F32, name=f"o{k}", tag="o", bufs=3)
                nc.vector.tensor_scalar_mul(o_sb[:, 0:512], o_ps[0][:], linv[:])
                nc.scalar.activation(o_sb[:, 512:1024], o_ps[1][:], AF.Copy,
                                     scale=linv[:])
                nc.sync.dma_start(out[(k - 1) * 128:k * 128, :], o_sb[:])
    if split:
        _split_multi_waits(nc)
    return nc


def _masks():
    j = np.arange(256)[None, :]
    i = np.arange(128)[:, None]
    mask0 = np.where(j <= i, 0.0, MASKVAL).astype(np.float32)
    mask1 = np.where(j <= 128 + i, 0.0, MASKVAL).astype(np.float32)
    return mask0, mask1


def _in_maps(x, w_q, w_k, w_v):
    bf = ml_dtypes.bfloat16
    x = np.asarray(x, np.float32)
    wqT = np.ascontiguousarray(np.asarray(w_q, np.float32).T).astype(bf)
    wkT = np.ascontiguousarray(np.asarray(w_k, np.float32).T).astype(bf)
    wvT = np.ascontiguousarray(np.asarray(w_v, np.float32).T).astype(bf)
    mask0, mask1 = _masks()

    in_maps = []
    for c in range(NCORES):
        b, p = divmod(c, 2)
        xb = x[b]                                    # [S, E]
        xoT = np.ascontiguousarray(xb[p * HT:(p + 1) * HT, :].T).astype(bf)
        qrows = np.concatenate(
            [xb[128 * (2 * (k - 1) + p):128 * (2 * (k - 1) + p) + 128, :]
             for k in range(1, NSLOT + 1)], axis=0)  # [NQ, E]
        xqT = np.ascontiguousarray(qrows.T).astype(bf)
        in_maps.append({
            "xoT": xoT, "xqT": xqT,
            "wqT": wqT, "wkT": wkT, "wvT": wvT,
            "mask": mask0 if p == 0 else mask1,
        })
    return in_maps


def _scatter(per_core_out):
    out = np.empty((B, S, D), dtype=np.float32)
    for c in range(NCORES):
        b, p = divmod(c, 2)
        oc = per_core_out[c]                         # [NQ, D]
        for k in range(1, NSLOT + 1):
            g = 2 * (k - 1) + p
            out[b, 128 * g:128 * (g + 1), :] = oc[128 * (k - 1):128 * k, :]
    return out


def kernel(x, w_q, w_k, w_v):
    global _prog
    if _prog is None:
        _prog = _build()
    in_maps = _in_maps(x, w_q, w_k, w_v)
    res = run_bass_kernel_spmd(_prog, in_maps, list(range(NCORES)))
    return _scatter([res.results[c]["out"] for c in range(NCORES)])

